# revision 42
# baseline (speedup 1.0000x reference)
"""Single-head attention kernel for Trainium2 (Bass/Tile), 8 NeuronCores.

Problem: B=4, S=4096, D=1024, H=128 fp32.
    q,k,v = x @ W{q,k,v};  out = softmax(q k^T / sqrt(H)) @ v

Sharding: 8 cores = (batch b, KEY-half kh).  Each core computes PARTIAL
attention for all 4096 queries over its 2048 keys; the host combines the
two partial results per batch: out = (outT_0 + outT_1) / (l_0 + l_1)
(unnormalized value-sums and softmax denominators add across key shards).
The host permutes each core's x rows so its key rows come first and
transposes/casts to xT [D, S] fp16.  Query order follows the same
permutation; the host maps it back when combining.

fp16 everywhere on the matmul operands (2-byte operands stream at
1 col/cycle @ 2.4 GHz; fp8 was measured on the real inputs and fails the
accuracy gate in every placement).  fp32 accumulation in PSUM.

Steady state is paced by ScalarE exp (64 tiles x ~1.1us); the PE runs
scores TWO kb ahead of the attnT@v accumulation so the
exp->AV->scores->exp latency cycle never binds:
    S(kb) waits only on the score bank freed by exp(kb-2).
Projection blocks are emitted just-in-time inside the chunk loops,
alternating between two PSUM banks (the dedicated proj bank and the
l bank, idle until chunk end) so bank-evacuation copies overlap the
next block's matmuls.  Row-sums l: DVE pair-adds + a running fp16
accumulator; ONE ones-matmul per 512-half per chunk (PE cost 1024
cyc/chunk).  PSUM: 1 proj + 4 scores + 2 outT + 1 l = 8 banks.

Measured ~111-113us (run-to-run +-1.2us, ALL of it front-DMA-arrival
jitter; the post-exp0 schedule is deterministic to ns).  Budget:
~8.7us runtime startup, ~11.5us front (DMA-gated: wq+s0+s1+wk
= 2.5 MB at ~330 GB/s aggregate), ~85us chunk region (PE-bound at
~95% occupancy; 512-col matmuls at the 213ns floor, LDWEIGHTS
hidden), ~7us tail (evac + DMA + ~2.5us fixed close-out).  Dead ends
measured on HW: fp8 in any placement (3e-2..1e-1 rel err), DMA XBAR
transpose for v (+12us), PSUM-direct output DMA (forbidden), extra
dma_starts in the front (~+1.8us dispatch each), sparse/mistimed
warm-up (PE drops to 1.2 GHz after ~1us idle; 512-col dense warm-up
ending at data arrival is required).  Next lever if ever revisited:
core-pair remote_dma exchange of q-blocks 4-7 (~4us realized; needs
semaphore plumbing outside the Tile framework).
"""

import math

import numpy as np

import concourse.bacc as bacc
import concourse.mybir as mybir
import concourse.tile as tile
from concourse.bass_utils import run_bass_kernel_spmd

B, S, D, H = 4, 4096, 1024, 128
NCORES = 8
SK = S // 2  # keys per core (2048)
RB = 512  # rows per projection block
NRB = S // RB  # 8 query blocks
NKRB = SK // RB  # 4 key blocks
QC = 1024  # queries per attention chunk
NQC = S // QC  # 4 chunks
NKB = SK // 128  # 16 key blocks of 128
NDC = D // 128  # 8 contraction chunks

F32 = mybir.dt.float32
F16 = mybir.dt.float16

_CACHE = {}


def build_nc():
    nc = bacc.Bacc("TRN2", target_bir_lowering=False, debug=False)

    xt_d = nc.dram_tensor("xt", [D, S], F16, kind="ExternalInput")
    # weights host-preswizzled to [128, NDC*H]: row p, chunk c = W[c*128+p, :]
    wq_d = nc.dram_tensor("wq", [128, NDC * H], F16, kind="ExternalInput")
    wk_d = nc.dram_tensor("wk", [128, NDC * H], F16, kind="ExternalInput")
    wv_d = nc.dram_tensor("wv", [128, NDC * H], F16, kind="ExternalInput")
    ident_d = nc.dram_tensor("ident", [128, 128], F16, kind="ExternalInput")
    ones_d = nc.dram_tensor("ones", [128, 1], F16, kind="ExternalInput")
    # partial (key-shard) unnormalized out^T [h, q] and denominators l [1, q]
    outT_d = nc.dram_tensor("outT", [H, S], F32, kind="ExternalOutput")
    l_d = nc.dram_tensor("l", [1, S], F32, kind="ExternalOutput")

    scale = 1.0 / math.sqrt(H)

    with tile.TileContext(nc) as tc:
        with (
            tc.tile_pool(name="const", bufs=1) as constp,
            tc.tile_pool(name="persist", bufs=1) as persist,
            tc.tile_pool(name="attn", bufs=6) as attn_pool,
            tc.tile_pool(name="lsum", bufs=3) as lsum_pool,
            tc.tile_pool(name="fin", bufs=2) as fin_pool,
            tc.tile_pool(name="ps_p", bufs=1, space="PSUM") as ps_p,
            tc.tile_pool(name="ps_s", bufs=2, space="PSUM") as ps_s,
            tc.tile_pool(name="ps_o", bufs=1, space="PSUM") as ps_o,
        ):
            # ---- DMA, ordered for the critical path ----
            w_sb = {}
            for name in ("wq", "wk", "wv"):
                w_sb[name] = constp.tile([128, NDC, H], F16, name=f"{name}_sb")

            def load_w(name):
                nc.sync.dma_start(
                    w_sb[name][:],
                    {"wq": wq_d, "wk": wk_d, "wv": wv_d}[name]
                    .ap()
                    .rearrange("p (c h) -> p c h", c=NDC),
                )

            xt_sb = persist.tile([128, NDC, S], F16, name="xt_sb")

            def load_slice(g):
                nc.sync.dma_start(
                    xt_sb[:, :, g * RB : (g + 1) * RB],
                    xt_d.ap()[:, g * RB : (g + 1) * RB].rearrange(
                        "(c p) s -> p c s", p=128
                    ),
                )

            ident = constp.tile([128, 128], F16, name="ident_sb")
            ones = constp.tile([128, 1], F16, name="ones_sb")

            def load_half_slice(g, half):
                c0, c1 = half * (NDC // 2), (half + 1) * (NDC // 2)
                nc.sync.dma_start(
                    xt_sb[:, c0:c1, g * RB : (g + 1) * RB],
                    xt_d.ap()[
                        c0 * 128 : c1 * 128, g * RB : (g + 1) * RB
                    ].rearrange("(c p) s -> p c s", p=128),
                )

            # slices 0/1 split in half so q0's first accumulation matmuls
            # start as soon as the first 0.5 MB lands; wk BEFORE slice 1 so
            # k-g0 can project while slice 1 streams (front emission order
            # is q0, k-g0, q1)
            load_w("wq")
            load_half_slice(0, 0)
            load_half_slice(0, 1)
            load_w("wk")
            load_half_slice(1, 0)
            load_half_slice(1, 1)
            nc.sync.dma_start(ident[:], ident_d.ap())
            nc.sync.dma_start(ones[:], ones_d.ap())
            load_w("wv")
            for g in range(2, NRB):
                load_slice(g)

            # ---- persistent activations ----
            qt_sb = persist.tile([128, S], F16, name="qt_sb")  # [h, q] all q
            kt_sb = persist.tile([128, SK], F16, name="kt_sb")  # [h, k] own
            v_sb = persist.tile([128, NKB, H], F16, name="v_sb")  # own keys
            vt_sb = persist.tile([128, SK], F16, name="vt_sb")  # staging

            # HAM warm-up on a locally memset tile: starts as soon as the
            # engines come up (~6us), needs NO input DMA, and has the PE at
            # 2.4 GHz by the time wq + xt slice 0 land.  ScalarE preloads the
            # exp table (reads the memset tile too -- also DMA-free).
            warm_src = constp.tile([128, 512], F16, name="warm_src")
            nc.gpsimd.memset(warm_src[:], 0.0)
            warm = constp.tile([1, 1], F32, name="warm_sb")
            nc.scalar.activation(
                warm[:], warm_src[0:1, 0:1], mybir.ActivationFunctionType.Exp
            )
            # Dense 512-col warm-up ending ~12us, when wq + slice0-half0
            # land (measured).  128-col warm-up matmuls never lift the PE
            # clock past 1.2 GHz (the DVFS wants ~3us of high-duty
            # streaming); with sparse warm-up the first ~13 real matmuls
            # run at 427ns instead of 216ns.
            warm_ps = ps_p.tile([128, 512], F32, tag="proj")
            for i in range(10):
                nc.tensor.matmul(
                    warm_ps[:],
                    warm_src[:, 0:128],
                    warm_src[:, 0:512],
                    start=(i == 0),
                    stop=(i == 9),
                )

            # Projections alternate between two PSUM banks so the bank
            # evacuation copy overlaps the next block's matmuls.  Slot A is
            # the dedicated proj bank; slot B is the l bank (idle until each
            # chunk's end).
            proj_slot = [0]

            def project(wname, dst_sb, rb):
                """One 512-row projection block through one PSUM bank."""
                if proj_slot[0] == 0:
                    ps = ps_p.tile([128, RB], F32, tag="proj")
                else:
                    ps = ps_o.tile([128, RB], F32, tag="l")
                proj_slot[0] ^= 1
                for dc in range(NDC):
                    nc.tensor.matmul(
                        ps[:, 0:RB],
                        w_sb[wname][:, dc, :],
                        xt_sb[:, dc, rb * RB : (rb + 1) * RB],
                        start=(dc == 0),
                        stop=(dc == NDC - 1),
                    )
                nc.vector.tensor_copy(dst_sb[:, rb * RB : (rb + 1) * RB], ps[:, 0:RB])

            def v_transpose(g):
                if proj_slot[0] == 0:
                    v_ps = ps_p.tile([128, RB], F16, tag="proj")
                else:
                    v_ps = ps_o.tile([128, RB], F16, tag="l")
                proj_slot[0] ^= 1
                for s in range(4):
                    nc.tensor.transpose(
                        v_ps[:, s * 128 : (s + 1) * 128],
                        vt_sb[:, g * RB + s * 128 : g * RB + (s + 1) * 128],
                        ident[:],
                    )
                nc.vector.tensor_copy(
                    v_sb[:, g * 4 : (g + 1) * 4, :].rearrange("p a b -> p (a b)"),
                    v_ps[:, 0 : 4 * H],
                )

            # Minimal front: exactly what S(0)/S(1) of chunk 0 need.
            # k-g0 between the q blocks: it only needs wk + slice 0, so the
            # PE projects it while slice 1 is still streaming in.
            project("wq", qt_sb, 0)
            project("wk", kt_sb, 0)
            project("wq", qt_sb, 1)

            # Just-in-time emission schedule for the remaining projection
            # blocks: chunk -> {kb slot -> [thunk]}.  k-group g must precede
            # S(4g); v-group g precedes A(4g) (which trails S by 2 slots);
            # q blocks 2c,2c+1 must finish before chunk c starts.
            def mk(f, *a):
                return lambda: f(*a)

            jit = {
                0: {
                    2: [mk(project, "wv", vt_sb, 0), mk(v_transpose, 0)],
                    3: [mk(project, "wk", kt_sb, 1)],
                    5: [mk(project, "wv", vt_sb, 1), mk(v_transpose, 1)],
                    7: [mk(project, "wk", kt_sb, 2)],
                    9: [mk(project, "wv", vt_sb, 2), mk(v_transpose, 2)],
                    11: [mk(project, "wk", kt_sb, 3)],
                    12: [mk(project, "wq", qt_sb, 2)],
                    13: [mk(project, "wv", vt_sb, 3), mk(v_transpose, 3)],
                    14: [mk(project, "wq", qt_sb, 3)],
                },
                1: {
                    5: [mk(project, "wq", qt_sb, 4)],
                    11: [mk(project, "wq", qt_sb, 5)],
                },
                2: {
                    5: [mk(project, "wq", qt_sb, 6)],
                    11: [mk(project, "wq", qt_sb, 7)],
                },
                3: {},
            }

            # ---- attention: scores run 2 kb ahead of AV ----
            # Chunk c's l-finalization (ones-matmuls + evacuation) is DEFERRED
            # into chunk c+1 at kb slot 3: the PE queue is in-order, and the
            # l-matmuls would otherwise stall the next chunk's first scores
            # behind the DVE accumulator chain (~2.2us bubble per boundary).
            prev_l = [None]  # final acc tile of the previous chunk

            def finalize_l(qc_prev):
                l_ps = ps_o.tile([64, 512], F32, tag="l")
                for h in range(QC // 512):
                    nc.tensor.matmul(
                        l_ps[h * 32 : h * 32 + 1, :],
                        ones[:],
                        prev_l[0][:, h * 512 : (h + 1) * 512],
                        start=True,
                        stop=True,
                        tile_position=(0, h * 32),
                    )
                l_sb = fin_pool.tile([1, QC], F32, tag="l_sb")
                nc.vector.tensor_copy(l_sb[:, 0:512], l_ps[0:1, :])
                nc.vector.tensor_copy(l_sb[:, 512:1024], l_ps[32:33, :])
                nc.sync.dma_start(
                    l_d.ap()[:, qc_prev * QC : (qc_prev + 1) * QC], l_sb[:]
                )

            # Chunk c's AV tail (A14/A15), outT evacuation and trailing pair
            # adds are ALSO deferred: emitted after chunk c+1's first two
            # scores, so exp(0') follows exp(15) with no PE-queue bubble.
            prev_st = [None]

            def chunk_tail(qc_prev):
                st = prev_st[0]
                st["av"](NKB - 2)
                st["av"](NKB - 1)
                st["pair"](NKB // 2 - 2)
                outT_sb = fin_pool.tile([128, QC], F32, tag="outT_sb")
                for h in range(QC // 512):
                    nc.vector.tensor_copy(
                        outT_sb[:, h * 512 : (h + 1) * 512],
                        st["outT"][:, h * 512 : (h + 1) * 512],
                    )
                    nc.sync.dma_start(
                        outT_d.ap()[
                            :, qc_prev * QC + h * 512 : qc_prev * QC + (h + 1) * 512
                        ],
                        outT_sb[:, h * 512 : (h + 1) * 512],
                    )
                st["pair"](NKB // 2 - 1)
                prev_l[0] = st["acc"][0]

            for qcidx in range(NQC):
                outT_ps = ps_o.tile([128, QC], F32, tag="outT")
                at_tiles = {}
                acc = [None]  # running fp16 row-sum accumulator

                def score(kb):
                    st_ps = ps_s.tile([128, QC], F32, tag="st")
                    for h in range(QC // 512):
                        nc.tensor.matmul(
                            st_ps[:, h * 512 : (h + 1) * 512],
                            kt_sb[:, kb * 128 : (kb + 1) * 128],
                            qt_sb[
                                :, qcidx * QC + h * 512 : qcidx * QC + (h + 1) * 512
                            ],
                            start=True,
                            stop=True,
                        )
                    at_sb = attn_pool.tile([128, QC], F16, tag="at")
                    nc.scalar.activation(
                        at_sb[:],
                        st_ps[:],
                        mybir.ActivationFunctionType.Exp,
                        scale=scale,
                    )
                    at_tiles[kb] = at_sb

                def accum_av(kb, at_tiles=at_tiles, outT_ps=outT_ps):
                    at_sb = at_tiles[kb]
                    for h in range(QC // 512):
                        nc.tensor.matmul(
                            outT_ps[:, h * 512 : (h + 1) * 512],
                            v_sb[:, kb, :],
                            at_sb[:, h * 512 : (h + 1) * 512],
                            start=(kb == 0),
                            stop=(kb == NKB - 1),
                        )

                def pair_acc(p, at_tiles=at_tiles, acc=acc):
                    """pair = at[2p]+at[2p+1]; acc += pair (fp16, DVE)."""
                    pair = lsum_pool.tile([128, QC], F16, tag="pair", bufs=3)
                    nc.vector.tensor_add(
                        pair[:], at_tiles[2 * p][:], at_tiles[2 * p + 1][:]
                    )
                    if acc[0] is None:
                        acc[0] = pair
                    else:
                        new = lsum_pool.tile([128, QC], F16, tag="acc", bufs=3)
                        nc.vector.tensor_add(new[:], acc[0][:], pair[:])
                        acc[0] = new

                score(0)
                score(1)
                if qcidx > 0:
                    chunk_tail(qcidx - 1)
                for kb in range(2, NKB):
                    score(kb)
                    # jit projections BEFORE the AV that may consume them:
                    # the dependency tracker follows emission order
                    for thunk in jit[qcidx].get(kb, []):
                        thunk()
                    if kb == 3 and qcidx > 0:
                        finalize_l(qcidx - 1)
                    accum_av(kb - 2)
                    if kb % 2 == 0 and kb >= 4:
                        pair_acc(kb // 2 - 2)
                prev_st[0] = {
                    "av": accum_av,
                    "pair": pair_acc,
                    "outT": outT_ps,
                    "acc": acc,
                    "at": at_tiles,
                }

            # ---- last chunk's tail: AV finish, evacuation, direct l ----
            st = prev_st[0]
            st["av"](NKB - 2)
            st["av"](NKB - 1)
            st["pair"](NKB // 2 - 2)
            # outT copies on ScalarE (idle after the final exp); DVE keeps
            # the l path
            outT_sb = fin_pool.tile([128, QC], F32, tag="outT_sb")
            for h in range(QC // 512):
                nc.scalar.copy(
                    outT_sb[:, h * 512 : (h + 1) * 512],
                    st["outT"][:, h * 512 : (h + 1) * 512],
                )
                nc.sync.dma_start(
                    outT_d.ap()[
                        :, (NQC - 1) * QC + h * 512 : (NQC - 1) * QC + (h + 1) * 512
                    ],
                    outT_sb[:, h * 512 : (h + 1) * 512],
                )
            # fold at[14], at[15] straight into the ones-matmul accumulation
            # group: no trailing pair/acc DVE chain after the final exp
            l_ps = ps_o.tile([64, 512], F32, tag="l")
            for h in range(QC // 512):
                lo, hi = h * 512, (h + 1) * 512
                for j, src in enumerate(
                    (st["acc"][0], st["at"][NKB - 2], st["at"][NKB - 1])
                ):
                    nc.tensor.matmul(
                        l_ps[h * 32 : h * 32 + 1, :],
                        ones[:],
                        src[:, lo:hi],
                        start=(j == 0),
                        stop=(j == 2),
                        tile_position=(0, h * 32),
                    )
            l_sb = fin_pool.tile([1, QC], F32, tag="l_sb")
            nc.vector.tensor_copy(l_sb[:, 0:512], l_ps[0:1, :])
            nc.vector.tensor_copy(l_sb[:, 512:1024], l_ps[32:33, :])
            nc.sync.dma_start(l_d.ap()[:, (NQC - 1) * QC : NQC * QC], l_sb[:])

    nc.compile()
    return nc


def _get_nc():
    if "nc" not in _CACHE:
        _CACHE["nc"] = build_nc()
    return _CACHE["nc"]


def _swizzle_w(W):
    # [D, H] -> [128, NDC*H]: row p, chunk c holds W[c*128+p, :]
    W = np.asarray(W, dtype=np.float16)
    return np.ascontiguousarray(
        W.reshape(NDC, 128, H).transpose(1, 0, 2).reshape(128, NDC * H)
    )


def make_in_maps(inputs, Wq, Wk, Wv):
    inputs = np.asarray(inputs, dtype=np.float32)
    Wq = _swizzle_w(Wq)
    Wk = _swizzle_w(Wk)
    Wv = _swizzle_w(Wv)
    ident = np.eye(128, dtype=np.float16)
    ones = np.ones((128, 1), dtype=np.float16)

    in_maps = []
    for c in range(NCORES):
        b, kh = divmod(c, 2)
        xb = inputs[b]
        # own key-half rows first; queries follow the same permutation
        xk = np.concatenate(
            [xb[kh * SK : (kh + 1) * SK], xb[(1 - kh) * SK : (2 - kh) * SK]], axis=0
        )
        xt = np.ascontiguousarray(xk.T.astype(np.float16))  # [D, S] fp16
        in_maps.append(
            {
                "xt": xt,
                "wq": Wq,
                "wk": Wk,
                "wv": Wv,
                "ident": ident,
                "ones": ones,
            }
        )
    return in_maps


def kernel(inputs, Wq, Wk, Wv):
    nc = _get_nc()
    in_maps = make_in_maps(inputs, Wq, Wk, Wv)

    res = run_bass_kernel_spmd(nc, in_maps, core_ids=list(range(NCORES)))

    out = np.empty((B, S, H), dtype=np.float32)
    for b in range(B):
        num = np.zeros((H, S), dtype=np.float32)
        den = np.zeros((1, S), dtype=np.float32)
        for kh in range(2):
            c = 2 * b + kh
            outT = res.results[c]["outT"]  # [H, S], query order permuted
            l = res.results[c]["l"]  # [1, S]
            # queries were ordered [kh-half, other-half]; map back
            perm = np.concatenate(
                [
                    np.arange(kh * SK, (kh + 1) * SK),
                    np.arange((1 - kh) * SK, (2 - kh) * SK),
                ]
            )
            num[:, perm] += outT
            den[:, perm] += l
        out[b] = (num / den).T
    return out


# revision 43
# speedup vs baseline: 1.1846x; 1.1846x over previous
"""Single-head attention kernel for Trainium2 (Bass/Tile), 8 NeuronCores.

Problem: B=4, S=4096, D=1024, H=128 fp32.
    q,k,v = x @ W{q,k,v};  out = softmax(q k^T / sqrt(H)) @ v

Sharding: 8 cores = (batch b, KEY-half kh).  Each core computes PARTIAL
attention for all 4096 queries over its 2048 keys; the host combines the
two partial results per batch: out = (outT_0 + outT_1) / (l_0 + l_1)
(unnormalized value-sums and softmax denominators add across key shards).
The host permutes each core's x rows so its key rows come first and
transposes/casts to xT [D, S] fp16.  Query order follows the same
permutation; the host maps it back when combining.

fp16 everywhere on the matmul operands (2-byte operands stream at
1 col/cycle @ 2.4 GHz; fp8 was measured on the real inputs and fails the
accuracy gate in every placement).  fp32 accumulation in PSUM.

Steady state is paced by ScalarE exp (64 tiles x ~1.1us); the PE runs
scores TWO kb ahead of the attnT@v accumulation so the
exp->AV->scores->exp latency cycle never binds:
    S(kb) waits only on the score bank freed by exp(kb-2).
Projection blocks are emitted just-in-time inside the chunk loops,
alternating between two PSUM banks (the dedicated proj bank and the
l bank, idle until chunk end) so bank-evacuation copies overlap the
next block's matmuls.  Row-sums l: DVE pair-adds + a running fp16
accumulator; ONE ones-matmul per 512-half per chunk (PE cost 1024
cyc/chunk).  PSUM: 1 proj + 4 scores + 2 outT + 1 l = 8 banks.

Measured ~111-113us (run-to-run +-1.2us, ALL of it front-DMA-arrival
jitter; the post-exp0 schedule is deterministic to ns).  Budget:
~8.7us runtime startup, ~11.5us front (DMA-gated: wq+s0+s1+wk
= 2.5 MB at ~330 GB/s aggregate), ~85us chunk region (PE-bound at
~95% occupancy; 512-col matmuls at the 213ns floor, LDWEIGHTS
hidden), ~7us tail (evac + DMA + ~2.5us fixed close-out).  Dead ends
measured on HW: fp8 in any placement (3e-2..1e-1 rel err), DMA XBAR
transpose for v (+12us), PSUM-direct output DMA (forbidden), extra
dma_starts in the front (~+1.8us dispatch each), sparse/mistimed
warm-up (PE drops to 1.2 GHz after ~1us idle; 512-col dense warm-up
ending at data arrival is required).  Next lever if ever revisited:
core-pair remote_dma exchange of q-blocks 4-7 (~4us realized; needs
semaphore plumbing outside the Tile framework).
"""

import math

import numpy as np

import concourse.bacc as bacc
import concourse.mybir as mybir
import concourse.tile as tile
from concourse.bass_utils import run_bass_kernel_spmd

B, S, D, H = 4, 4096, 1024, 128
NCORES = 8
SK = S // 2  # keys per core (2048)
RB = 512  # rows per projection block
NRB = S // RB  # 8 query blocks
NKRB = SK // RB  # 4 key blocks
QC = 1024  # queries per attention chunk
NQC = S // QC  # 4 chunks
NKB = SK // 128  # 16 key blocks of 128
NDC = D // 128  # 8 contraction chunks

F32 = mybir.dt.float32
F16 = mybir.dt.float16

_CACHE = {}


def build_nc():
    nc = bacc.Bacc("TRN2", target_bir_lowering=False, debug=False)

    xt_d = nc.dram_tensor("xt", [D, S], F16, kind="ExternalInput")
    # weights host-preswizzled to [128, NDC*H]: row p, chunk c = W[c*128+p, :]
    wq_d = nc.dram_tensor("wq", [128, NDC * H], F16, kind="ExternalInput")
    wk_d = nc.dram_tensor("wk", [128, NDC * H], F16, kind="ExternalInput")
    wv_d = nc.dram_tensor("wv", [128, NDC * H], F16, kind="ExternalInput")
    ident_d = nc.dram_tensor("ident", [128, 128], F16, kind="ExternalInput")
    ones_d = nc.dram_tensor("ones", [128, 1], F16, kind="ExternalInput")
    # partial (key-shard) unnormalized out^T [h, q] and denominators l [1, q]
    outT_d = nc.dram_tensor("outT", [H, S], F32, kind="ExternalOutput")
    l_d = nc.dram_tensor("l", [1, S], F32, kind="ExternalOutput")

    scale = 1.0 / math.sqrt(H)

    with tile.TileContext(nc) as tc:
        with (
            tc.tile_pool(name="const", bufs=1) as constp,
            tc.tile_pool(name="persist", bufs=1) as persist,
            tc.tile_pool(name="attn", bufs=6) as attn_pool,
            tc.tile_pool(name="lsum", bufs=3) as lsum_pool,
            tc.tile_pool(name="fin", bufs=2) as fin_pool,
            tc.tile_pool(name="ps_p", bufs=1, space="PSUM") as ps_p,
            tc.tile_pool(name="ps_s", bufs=2, space="PSUM") as ps_s,
            tc.tile_pool(name="ps_o", bufs=1, space="PSUM") as ps_o,
        ):
            # ---- DMA, ordered for the critical path ----
            w_sb = {}
            for name in ("wq", "wk", "wv"):
                w_sb[name] = constp.tile([128, NDC, H], F16, name=f"{name}_sb")

            def load_w(name):
                nc.sync.dma_start(
                    w_sb[name][:],
                    {"wq": wq_d, "wk": wk_d, "wv": wv_d}[name]
                    .ap()
                    .rearrange("p (c h) -> p c h", c=NDC),
                )

            xt_sb = persist.tile([128, NDC, S], F16, name="xt_sb")

            def load_slice(g):
                nc.sync.dma_start(
                    xt_sb[:, :, g * RB : (g + 1) * RB],
                    xt_d.ap()[:, g * RB : (g + 1) * RB].rearrange(
                        "(c p) s -> p c s", p=128
                    ),
                )

            ident = constp.tile([128, 128], F16, name="ident_sb")
            ones = constp.tile([128, 1], F16, name="ones_sb")

            def load_half_slice(g, half):
                c0, c1 = half * (NDC // 2), (half + 1) * (NDC // 2)
                nc.sync.dma_start(
                    xt_sb[:, c0:c1, g * RB : (g + 1) * RB],
                    xt_d.ap()[
                        c0 * 128 : c1 * 128, g * RB : (g + 1) * RB
                    ].rearrange("(c p) s -> p c s", p=128),
                )

            # slices 0/1 split in half so q0's first accumulation matmuls
            # start as soon as the first 0.5 MB lands; wk BEFORE slice 1 so
            # k-g0 can project while slice 1 streams (front emission order
            # is q0, k-g0, q1)
            load_w("wq")
            load_half_slice(0, 0)
            load_half_slice(0, 1)
            load_w("wk")
            load_half_slice(1, 0)
            load_half_slice(1, 1)
            nc.sync.dma_start(ident[:], ident_d.ap())
            nc.sync.dma_start(ones[:], ones_d.ap())
            load_w("wv")
            for g in range(2, NRB):
                load_slice(g)

            # ---- persistent activations ----
            qt_sb = persist.tile([128, S], F16, name="qt_sb")  # [h, q] all q
            kt_sb = persist.tile([128, SK], F16, name="kt_sb")  # [h, k] own
            v_sb = persist.tile([128, NKB, H], F16, name="v_sb")  # own keys
            vt_sb = persist.tile([128, SK], F16, name="vt_sb")  # staging

            # HAM warm-up on a locally memset tile: starts as soon as the
            # engines come up (~6us), needs NO input DMA, and has the PE at
            # 2.4 GHz by the time wq + xt slice 0 land.  ScalarE preloads the
            # exp table (reads the memset tile too -- also DMA-free).
            warm_src = constp.tile([128, 512], F16, name="warm_src")
            nc.gpsimd.memset(warm_src[:], 0.0)
            warm = constp.tile([1, 1], F32, name="warm_sb")
            nc.scalar.activation(
                warm[:], warm_src[0:1, 0:1], mybir.ActivationFunctionType.Exp
            )
            # Dense 512-col warm-up ending ~12us, when wq + slice0-half0
            # land (measured).  128-col warm-up matmuls never lift the PE
            # clock past 1.2 GHz (the DVFS wants ~3us of high-duty
            # streaming); with sparse warm-up the first ~13 real matmuls
            # run at 427ns instead of 216ns.
            warm_ps = ps_p.tile([128, 512], F32, tag="proj")
            for i in range(10):
                nc.tensor.matmul(
                    warm_ps[:],
                    warm_src[:, 0:128],
                    warm_src[:, 0:512],
                    start=(i == 0),
                    stop=(i == 9),
                )

            # Projections alternate between two PSUM banks so the bank
            # evacuation copy overlaps the next block's matmuls.  Slot A is
            # the dedicated proj bank; slot B is the l bank (idle until each
            # chunk's end).
            proj_slot = [0]

            def project(wname, dst_sb, rb):
                """One 512-row projection block through one PSUM bank."""
                if proj_slot[0] == 0:
                    ps = ps_p.tile([128, RB], F32, tag="proj")
                else:
                    ps = ps_o.tile([128, RB], F32, tag="l")
                proj_slot[0] ^= 1
                for dc in range(NDC):
                    nc.tensor.matmul(
                        ps[:, 0:RB],
                        w_sb[wname][:, dc, :],
                        xt_sb[:, dc, rb * RB : (rb + 1) * RB],
                        start=(dc == 0),
                        stop=(dc == NDC - 1),
                    )
                nc.vector.tensor_copy(dst_sb[:, rb * RB : (rb + 1) * RB], ps[:, 0:RB])

            def v_transpose(g):
                if proj_slot[0] == 0:
                    v_ps = ps_p.tile([128, RB], F16, tag="proj")
                else:
                    v_ps = ps_o.tile([128, RB], F16, tag="l")
                proj_slot[0] ^= 1
                for s in range(4):
                    nc.tensor.transpose(
                        v_ps[:, s * 128 : (s + 1) * 128],
                        vt_sb[:, g * RB + s * 128 : g * RB + (s + 1) * 128],
                        ident[:],
                    )
                nc.vector.tensor_copy(
                    v_sb[:, g * 4 : (g + 1) * 4, :].rearrange("p a b -> p (a b)"),
                    v_ps[:, 0 : 4 * H],
                )

            # Minimal front: exactly what S(0)/S(1) of chunk 0 need.
            # k-g0 between the q blocks: it only needs wk + slice 0, so the
            # PE projects it while slice 1 is still streaming in.
            project("wq", qt_sb, 0)
            project("wk", kt_sb, 0)
            project("wq", qt_sb, 1)

            # Just-in-time emission schedule for the remaining projection
            # blocks: chunk -> {kb slot -> [thunk]}.  k-group g must precede
            # S(4g); v-group g precedes A(4g) (which trails S by 2 slots);
            # q blocks 2c,2c+1 must finish before chunk c starts.
            def mk(f, *a):
                return lambda: f(*a)

            jit = {
                0: {
                    2: [mk(project, "wv", vt_sb, 0), mk(v_transpose, 0)],
                    3: [mk(project, "wk", kt_sb, 1)],
                    5: [mk(project, "wv", vt_sb, 1), mk(v_transpose, 1)],
                    7: [mk(project, "wk", kt_sb, 2)],
                    9: [mk(project, "wv", vt_sb, 2), mk(v_transpose, 2)],
                    11: [mk(project, "wk", kt_sb, 3)],
                    12: [mk(project, "wq", qt_sb, 2)],
                    13: [mk(project, "wv", vt_sb, 3), mk(v_transpose, 3)],
                    14: [mk(project, "wq", qt_sb, 3)],
                },
                1: {
                    5: [mk(project, "wq", qt_sb, 4)],
                    11: [mk(project, "wq", qt_sb, 5)],
                },
                2: {
                    5: [mk(project, "wq", qt_sb, 6)],
                    11: [mk(project, "wq", qt_sb, 7)],
                },
                3: {},
            }

            # ---- attention: scores run 2 kb ahead of AV ----
            # Chunk c's l-finalization (ones-matmuls + evacuation) is DEFERRED
            # into chunk c+1 at kb slot 3: the PE queue is in-order, and the
            # l-matmuls would otherwise stall the next chunk's first scores
            # behind the DVE accumulator chain (~2.2us bubble per boundary).
            prev_l = [None]  # final acc tile of the previous chunk

            def finalize_l(qc_prev):
                l_ps = ps_o.tile([64, 512], F32, tag="l")
                for h in range(QC // 512):
                    nc.tensor.matmul(
                        l_ps[h * 32 : h * 32 + 1, :],
                        ones[:],
                        prev_l[0][:, h * 512 : (h + 1) * 512],
                        start=True,
                        stop=True,
                        tile_position=(0, h * 32),
                    )
                l_sb = fin_pool.tile([1, QC], F32, tag="l_sb")
                nc.vector.tensor_copy(l_sb[:, 0:512], l_ps[0:1, :])
                nc.vector.tensor_copy(l_sb[:, 512:1024], l_ps[32:33, :])
                nc.sync.dma_start(
                    l_d.ap()[:, qc_prev * QC : (qc_prev + 1) * QC], l_sb[:]
                )

            # Chunk c's AV tail (A14/A15), outT evacuation and trailing pair
            # adds are ALSO deferred: emitted after chunk c+1's first two
            # scores, so exp(0') follows exp(15) with no PE-queue bubble.
            prev_st = [None]

            def chunk_tail(qc_prev):
                st = prev_st[0]
                st["av"](NKB - 2)
                st["av"](NKB - 1)
                st["pair"](NKB // 2 - 2)
                outT_sb = fin_pool.tile([128, QC], F32, tag="outT_sb")
                for h in range(QC // 512):
                    nc.vector.tensor_copy(
                        outT_sb[:, h * 512 : (h + 1) * 512],
                        st["outT"][:, h * 512 : (h + 1) * 512],
                    )
                    nc.sync.dma_start(
                        outT_d.ap()[
                            :, qc_prev * QC + h * 512 : qc_prev * QC + (h + 1) * 512
                        ],
                        outT_sb[:, h * 512 : (h + 1) * 512],
                    )
                st["pair"](NKB // 2 - 1)
                prev_l[0] = st["acc"][0]

            for qcidx in range(NQC):
                outT_ps = ps_o.tile([128, QC], F32, tag="outT")
                at_tiles = {}
                acc = [None]  # running fp16 row-sum accumulator

                def score(kb):
                    st_ps = ps_s.tile([128, QC], F32, tag="st")
                    for h in range(QC // 512):
                        nc.tensor.matmul(
                            st_ps[:, h * 512 : (h + 1) * 512],
                            kt_sb[:, kb * 128 : (kb + 1) * 128],
                            qt_sb[
                                :, qcidx * QC + h * 512 : qcidx * QC + (h + 1) * 512
                            ],
                            start=True,
                            stop=True,
                        )
                    at_sb = attn_pool.tile([128, QC], F16, tag="at")
                    nc.scalar.activation(
                        at_sb[:],
                        st_ps[:],
                        mybir.ActivationFunctionType.Exp,
                        scale=scale,
                    )
                    at_tiles[kb] = at_sb

                def accum_av(kb, at_tiles=at_tiles, outT_ps=outT_ps):
                    at_sb = at_tiles[kb]
                    for h in range(QC // 512):
                        nc.tensor.matmul(
                            outT_ps[:, h * 512 : (h + 1) * 512],
                            v_sb[:, kb, :],
                            at_sb[:, h * 512 : (h + 1) * 512],
                            start=(kb == 0),
                            stop=(kb == NKB - 1),
                        )

                def pair_acc(p, at_tiles=at_tiles, acc=acc):
                    """pair = at[2p]+at[2p+1]; acc += pair (fp16, DVE)."""
                    pair = lsum_pool.tile([128, QC], F16, tag="pair", bufs=3)
                    nc.vector.tensor_add(
                        pair[:], at_tiles[2 * p][:], at_tiles[2 * p + 1][:]
                    )
                    if acc[0] is None:
                        acc[0] = pair
                    else:
                        new = lsum_pool.tile([128, QC], F16, tag="acc", bufs=3)
                        nc.vector.tensor_add(new[:], acc[0][:], pair[:])
                        acc[0] = new

                score(0)
                score(1)
                if qcidx > 0:
                    chunk_tail(qcidx - 1)
                for kb in range(2, NKB):
                    score(kb)
                    # jit projections BEFORE the AV that may consume them:
                    # the dependency tracker follows emission order
                    for thunk in jit[qcidx].get(kb, []):
                        thunk()
                    if kb == 3 and qcidx > 0:
                        finalize_l(qcidx - 1)
                    accum_av(kb - 2)
                    if kb % 2 == 0 and kb >= 4:
                        pair_acc(kb // 2 - 2)
                prev_st[0] = {
                    "av": accum_av,
                    "pair": pair_acc,
                    "outT": outT_ps,
                    "acc": acc,
                    "at": at_tiles,
                }

            # ---- last chunk's tail: AV finish, evacuation, direct l ----
            st = prev_st[0]
            st["pair"](NKB // 2 - 2)
            # fold at[14], at[15] straight into the ones-matmul accumulation
            # group, SPLIT around the AV tail: the acc6/at14 matmuls only
            # need exp(13)/exp(14), so emitting them before A14/A15 leaves
            # just two 512-col matmuls on the post-exp(15) chain
            l_ps = ps_o.tile([64, 512], F32, tag="l")

            def l_mm(src, start, stop):
                for h in range(QC // 512):
                    nc.tensor.matmul(
                        l_ps[h * 32 : h * 32 + 1, :],
                        ones[:],
                        src[:, h * 512 : (h + 1) * 512],
                        start=start,
                        stop=stop,
                        tile_position=(0, h * 32),
                    )

            l_mm(st["acc"][0], True, False)
            l_mm(st["at"][NKB - 2], False, False)
            st["av"](NKB - 2)
            st["av"](NKB - 1)
            l_mm(st["at"][NKB - 1], False, True)
            # outT copies on ScalarE (idle after the final exp); DVE keeps
            # the l path
            outT_sb = fin_pool.tile([128, QC], F32, tag="outT_sb")
            for h in range(QC // 512):
                nc.scalar.copy(
                    outT_sb[:, h * 512 : (h + 1) * 512],
                    st["outT"][:, h * 512 : (h + 1) * 512],
                )
                nc.sync.dma_start(
                    outT_d.ap()[
                        :, (NQC - 1) * QC + h * 512 : (NQC - 1) * QC + (h + 1) * 512
                    ],
                    outT_sb[:, h * 512 : (h + 1) * 512],
                )
            l_sb = fin_pool.tile([1, QC], F32, tag="l_sb")
            nc.vector.tensor_copy(l_sb[:, 0:512], l_ps[0:1, :])
            nc.vector.tensor_copy(l_sb[:, 512:1024], l_ps[32:33, :])
            nc.sync.dma_start(l_d.ap()[:, (NQC - 1) * QC : NQC * QC], l_sb[:])

    nc.compile()
    return nc


def _get_nc():
    if "nc" not in _CACHE:
        _CACHE["nc"] = build_nc()
    return _CACHE["nc"]


def _swizzle_w(W):
    # [D, H] -> [128, NDC*H]: row p, chunk c holds W[c*128+p, :]
    W = np.asarray(W, dtype=np.float16)
    return np.ascontiguousarray(
        W.reshape(NDC, 128, H).transpose(1, 0, 2).reshape(128, NDC * H)
    )


def make_in_maps(inputs, Wq, Wk, Wv):
    inputs = np.asarray(inputs, dtype=np.float32)
    Wq = _swizzle_w(Wq)
    Wk = _swizzle_w(Wk)
    Wv = _swizzle_w(Wv)
    ident = np.eye(128, dtype=np.float16)
    ones = np.ones((128, 1), dtype=np.float16)

    in_maps = []
    for c in range(NCORES):
        b, kh = divmod(c, 2)
        xb = inputs[b]
        # own key-half rows first; queries follow the same permutation
        xk = np.concatenate(
            [xb[kh * SK : (kh + 1) * SK], xb[(1 - kh) * SK : (2 - kh) * SK]], axis=0
        )
        xt = np.ascontiguousarray(xk.T.astype(np.float16))  # [D, S] fp16
        in_maps.append(
            {
                "xt": xt,
                "wq": Wq,
                "wk": Wk,
                "wv": Wv,
                "ident": ident,
                "ones": ones,
            }
        )
    return in_maps


def kernel(inputs, Wq, Wk, Wv):
    nc = _get_nc()
    in_maps = make_in_maps(inputs, Wq, Wk, Wv)

    res = run_bass_kernel_spmd(nc, in_maps, core_ids=list(range(NCORES)))

    out = np.empty((B, S, H), dtype=np.float32)
    for b in range(B):
        num = np.zeros((H, S), dtype=np.float32)
        den = np.zeros((1, S), dtype=np.float32)
        for kh in range(2):
            c = 2 * b + kh
            outT = res.results[c]["outT"]  # [H, S], query order permuted
            l = res.results[c]["l"]  # [1, S]
            # queries were ordered [kh-half, other-half]; map back
            perm = np.concatenate(
                [
                    np.arange(kh * SK, (kh + 1) * SK),
                    np.arange((1 - kh) * SK, (2 - kh) * SK),
                ]
            )
            num[:, perm] += outT
            den[:, perm] += l
        out[b] = (num / den).T
    return out
